# revision 7
# baseline (speedup 1.0000x reference)
"""Multi-head-free attention kernel for Trainium2, SPMD across 8 NeuronCores.

Problem: x[4, 4096, 512] -> Q,K,V = x@W* + b* (d_head=64);
Z = softmax(Q K^T / 8) V  -> [4, 4096, 64]

Sharding: data-parallel over batch (4) x query-halves (2) = 8 cores.
Each core handles 2048 queries of one batch against all 4096 keys of
that batch.  The key/value rows are fed in rolled order so every core's
queries sit at rows 0..2047 of its input -- softmax(QK^T)V is invariant
to a permutation of the key axis, so the result is exact.

Device algorithm (per core), bf16 matmuls with f32 PSUM accumulation:
  - x^T arrives pre-transposed [512, 4096] (host layout prep), cast to bf16
  - Q^T[64, 2048], and a fused [V^T; K^T] projection [128, 4096]
  - V^T is PE-transposed into V-natural [k,64] tiles with a ones column
    appended -> [k, 65]
  - scores are computed TRANSPOSED: score^T[k, q] blocks via
    lhsT=K^T-block (contraction=64).  Pairs of k-blocks are row-packed
    onto partition groups 0-63 / 64-127 so two matmuls run concurrently.
  - exp on the scalar engine straight out of PSUM (scale=1/8 fused)
  - P^T @ [V|1] accumulates Z^T[64, q] AND the softmax denominator
    (row 64) in one PSUM tile across all 32 k-blocks
  - reciprocal + rank-1 broadcast matmul + elementwise multiply
  - output is Z^T [64, 2048] f32; the host transposes back.
"""

import os
import sys

import numpy as np

for _p in ("/opt/trn_rl_repo", "/root/.axon_site/_ro/trn_rl_repo"):
    if os.path.isdir(_p) and _p not in sys.path:
        sys.path.insert(0, _p)

import concourse.bass as bass
import concourse.mybir as mybir
from concourse import bacc
from concourse.bass_utils import run_bass_kernel_spmd
from concourse.masks import make_identity
from concourse.tile import TileContext

F32 = mybir.dt.float32
BF16 = mybir.dt.bfloat16

B = 4          # batch
S = 4096       # sequence (keys)
SQ = 2048      # queries per core
W = 512        # d_model
E = 64         # d_head
P = 128
WC = W // P    # 4 w-chunks
NQC = SQ // 512  # 4 query chunks of 512
NKB = S // P   # 32 key blocks of 128
NSC = S // 512  # 8 chunks for the KV projection

N_CORES = 8


def build_graph() -> bass.Bass:
    nc = bacc.Bacc(
        "TRN2", target_bir_lowering=False, debug=False, num_devices=N_CORES
    )

    xt_d = nc.declare_dram_parameter("xt", [W, S], F32, isOutput=False)
    wq_d = nc.declare_dram_parameter("wq", [W, E], F32, isOutput=False)
    # wvk packs [Wv | Wk] -> [512, 128]
    wvk_d = nc.declare_dram_parameter("wvk", [W, 2 * E], F32, isOutput=False)
    bq_d = nc.declare_dram_parameter("bq", [E], F32, isOutput=False)
    # bkv packs [bv; bk] -> [128]
    bkv_d = nc.declare_dram_parameter("bkv", [2 * E], F32, isOutput=False)
    out_d = nc.declare_dram_parameter("out", [E, SQ], F32, isOutput=True)

    xt_view = xt_d.rearrange("(c p) s -> c p s", p=P)

    with TileContext(nc) as tc:
        with (
            tc.tile_pool(name="consts", bufs=1) as consts,
            tc.tile_pool(name="persist", bufs=1) as persist,
        ):
            # --- constants ---
            id64 = consts.tile([E, E], BF16)
            make_identity(nc, id64)
            onesw = consts.tile([E + 1, E], F32)
            nc.gpsimd.memset(onesw[E : E + 1, :], 1.0)
            bq_t = consts.tile([E, 1], F32)
            nc.sync.dma_start(bq_t, bq_d[:, None])
            bkv_t = consts.tile([P, 1], F32)
            nc.sync.dma_start(bkv_t, bkv_d[:, None])

            # --- persistent activations ---
            xtb = persist.tile([P, WC, S], BF16)      # x^T bf16
            qt = persist.tile([P, SQ], BF16)          # Q^T duplicated on both halves
            kvt = persist.tile([P, S], BF16)          # rows 0:64 V^T, 64:128 K^T
            ktd = persist.tile([P, S], BF16)          # rows 0:64 K^T (copy)
            vnat = persist.tile([P, NKB, E + 1], BF16)  # V natural + ones col

            # --- phase A: load, cast, project ---
            with (
                tc.tile_pool(name="pa_sb", bufs=2) as pa_sb,
                tc.tile_pool(name="pa_ps", bufs=2, space="PSUM") as pa_ps,
                tc.tile_pool(name="pa_pst", bufs=2, space="PSUM") as pa_pst,
            ):
                # weights
                wqf = pa_sb.tile([P, WC, E], F32, tag="wqf")
                nc.sync.dma_start(wqf, wq_d.rearrange("(c p) e -> p c e", p=P))
                wq_b = consts.tile([P, WC, E], BF16)
                nc.vector.tensor_copy(wq_b, wqf)
                wvkf = pa_sb.tile([P, WC, 2 * E], F32, tag="wvkf")
                nc.sync.dma_start(wvkf, wvk_d.rearrange("(c p) e -> p c e", p=P))
                wvk_b = consts.tile([P, WC, 2 * E], BF16)
                nc.vector.tensor_copy(wvk_b, wvkf)

                # x^T load + cast (fine-grained so each cast waits on ONE DMA;
                # DMAs spread across queues, casts across engines)
                for wc in range(WC):
                    for hh in range(2):
                        half = S // 2
                        hs = slice(hh * half, (hh + 1) * half)
                        xf = pa_sb.tile([P, half], F32, tag=f"xf{wc}_{hh}")
                        nc.sync.dma_start(xf, xt_view[wc, :, hs])
                        eng = nc.vector if (2 * wc + hh) % 2 == 0 else nc.gpsimd
                        eng.tensor_copy(xtb[:, wc, hs], xf)

                # Q^T projection (+bias), then duplicate onto partitions 64:128
                for qc in range(NQC):
                    qp = pa_ps.tile([E, 512], F32, tag="qp")
                    for wc in range(WC):
                        nc.tensor.matmul(
                            qp,
                            wq_b[:, wc, :],
                            xtb[:, wc, qc * 512 : (qc + 1) * 512],
                            start=(wc == 0),
                            stop=(wc == WC - 1),
                        )
                    nc.vector.tensor_scalar_add(
                        qt[0:E, qc * 512 : (qc + 1) * 512], qp, bq_t
                    )
                nc.sync.dma_start(qt[E:P, :], qt[0:E, :])

                # fused [V^T; K^T] projection (+biases)
                for sc in range(NSC):
                    kvp = pa_ps.tile([P, 512], F32, tag="kvp")
                    for wc in range(WC):
                        nc.tensor.matmul(
                            kvp,
                            wvk_b[:, wc, :],
                            xtb[:, wc, sc * 512 : (sc + 1) * 512],
                            start=(wc == 0),
                            stop=(wc == WC - 1),
                        )
                    nc.vector.tensor_scalar_add(
                        kvt[:, sc * 512 : (sc + 1) * 512], kvp, bkv_t
                    )
                # K^T copy for the even (partition 0-63) score matmuls
                nc.sync.dma_start(ktd[0:E, :], kvt[E:P, :])

                # V natural [k, 64] + ones column via PE transpose
                nc.gpsimd.memset(vnat[:, :, E : E + 1], 1.0)
                for kb in range(NKB):
                    vps = pa_pst.tile([P, E], BF16, tag="vps")
                    nc.tensor.transpose(
                        vps, kvt[0:E, kb * P : (kb + 1) * P], id64
                    )
                    nc.vector.tensor_copy(vnat[:, kb, 0:E], vps)

            # --- phase B: flash attention sweep ---
            with (
                tc.tile_pool(name="sp", bufs=2, space="PSUM") as spP,
                tc.tile_pool(name="zp", bufs=2, space="PSUM") as zpP,
                tc.tile_pool(name="bc", bufs=1, space="PSUM") as bcP,
                tc.tile_pool(name="pexp", bufs=3) as peP,
                tc.tile_pool(name="fin", bufs=2) as finP,
            ):
                for qc in range(NQC):
                    qs = slice(qc * 512, (qc + 1) * 512)
                    zp = zpP.tile([E + 1, 512], F32, tag="zp")
                    for g in range(NKB // 2):
                        kb0, kb1 = 2 * g, 2 * g + 1
                        sp = spP.tile([P, 2, 512], F32, tag="sp")
                        # row-packed score pair: score^T blocks [128k, 512q]
                        nc.tensor.matmul(
                            sp[:, 0, :],
                            ktd[0:E, kb0 * P : (kb0 + 1) * P],
                            qt[0:E, qs],
                            start=True,
                            stop=True,
                        )
                        nc.tensor.matmul(
                            sp[:, 1, :],
                            kvt[E:P, kb1 * P : (kb1 + 1) * P],
                            qt[E:P, qs],
                            start=True,
                            stop=True,
                        )
                        pe = peP.tile([P, 2, 512], BF16, tag="pe")
                        nc.scalar.activation(
                            pe, sp, mybir.ActivationFunctionType.Exp, scale=0.125
                        )
                        nc.tensor.matmul(
                            zp,
                            vnat[:, kb0, :],
                            pe[:, 0, :],
                            start=(g == 0),
                            stop=False,
                        )
                        nc.tensor.matmul(
                            zp,
                            vnat[:, kb1, :],
                            pe[:, 1, :],
                            start=False,
                            stop=(g == NKB // 2 - 1),
                        )
                    # softmax denominator -> broadcast -> divide
                    rdt = finP.tile([E + 1, 512], F32, tag="rdt")
                    nc.vector.reciprocal(rdt[E : E + 1, :], zp[E : E + 1, :])
                    bc = bcP.tile([E, 512], F32, tag="bc")
                    nc.tensor.matmul(
                        bc, onesw[E : E + 1, :], rdt[E : E + 1, :],
                        start=True, stop=True,
                    )
                    bcs = finP.tile([E, 512], F32, tag="bcs")
                    nc.vector.tensor_copy(bcs, bc)
                    zf = finP.tile([E, 512], F32, tag="zf")
                    nc.vector.tensor_tensor(
                        zf, zp[0:E, :], bcs, mybir.AluOpType.mult
                    )
                    nc.sync.dma_start(out_d[:, qs], zf)

    nc.compile()
    return nc


_GRAPH_CACHE: bass.Bass | None = None


def _get_graph() -> bass.Bass:
    global _GRAPH_CACHE
    if _GRAPH_CACHE is None:
        _GRAPH_CACHE = build_graph()
    return _GRAPH_CACHE


def _make_in_maps(x, Wq, bq, Wk, bk, Wv, bv):
    x = np.asarray(x, dtype=np.float32)
    wq = np.ascontiguousarray(np.asarray(Wq, dtype=np.float32))
    wvk = np.ascontiguousarray(
        np.concatenate(
            [np.asarray(Wv, dtype=np.float32), np.asarray(Wk, dtype=np.float32)],
            axis=1,
        )
    )
    bq_ = np.ascontiguousarray(np.asarray(bq, dtype=np.float32))
    bkv = np.ascontiguousarray(
        np.concatenate(
            [np.asarray(bv, dtype=np.float32), np.asarray(bk, dtype=np.float32)]
        )
    )
    in_maps = []
    for c in range(N_CORES):
        b, h = divmod(c, 2)
        xl = np.roll(x[b], -h * SQ, axis=0)
        xt = np.ascontiguousarray(xl.T)
        in_maps.append(
            {"xt": xt, "wq": wq, "wvk": wvk, "bq": bq_, "bkv": bkv}
        )
    return in_maps


def _run(inputs: dict, trace: bool = False):
    nc = _get_graph()
    in_maps = _make_in_maps(**inputs)
    res = run_bass_kernel_spmd(
        nc, in_maps, core_ids=list(range(N_CORES)), trace=trace
    )
    out = np.zeros((B, S, E), dtype=np.float32)
    for c in range(N_CORES):
        b, h = divmod(c, 2)
        out[b, h * SQ : (h + 1) * SQ, :] = res.results[c]["out"].T
    return out, res


def kernel(**inputs) -> np.ndarray:
    out, _ = _run(inputs, trace=False)
    return out


# revision 10
# speedup vs baseline: 1.2099x; 1.2099x over previous
"""Multi-head-free attention kernel for Trainium2, SPMD across 8 NeuronCores.

Problem: x[4, 4096, 512] -> Q,K,V = x@W* + b* (d_head=64);
Z = softmax(Q K^T / 8) V  -> [4, 4096, 64]

Sharding: data-parallel over batch (4) x query-halves (2) = 8 cores.
Each core handles 2048 queries of one batch against all 4096 keys of
that batch.  The key/value rows are fed in rolled order so every core's
queries sit at rows 0..2047 of its input -- softmax(QK^T)V is invariant
to a permutation of the key axis, so the result is exact.

Device algorithm (per core), bf16 matmuls with f32 PSUM accumulation:
  - x^T arrives pre-transposed [512, 4096] (host layout prep), cast to bf16
  - Q^T[64, 2048], and a fused [V^T; K^T] projection [128, 4096]
  - V^T is PE-transposed into V-natural [k,64] tiles with a ones column
    appended -> [k, 65]
  - scores are computed TRANSPOSED: score^T[k, q] blocks via
    lhsT=K^T-block (contraction=64).  Pairs of k-blocks are row-packed
    onto partition groups 0-63 / 64-127 so two matmuls run concurrently.
  - exp on the scalar engine straight out of PSUM (scale=1/8 fused)
  - P^T @ [V|1] accumulates Z^T[64, q] AND the softmax denominator
    (row 64) in one PSUM tile across all 32 k-blocks
  - reciprocal + rank-1 broadcast matmul + elementwise multiply
  - output is Z^T [64, 2048] f32; the host transposes back.
"""

import os
import sys

import numpy as np

for _p in ("/opt/trn_rl_repo", "/root/.axon_site/_ro/trn_rl_repo"):
    if os.path.isdir(_p) and _p not in sys.path:
        sys.path.insert(0, _p)

import concourse.bass as bass
import concourse.mybir as mybir
from concourse import bacc
from concourse.bass_utils import run_bass_kernel_spmd
from concourse.masks import make_identity
from concourse.tile import TileContext

F32 = mybir.dt.float32
BF16 = mybir.dt.bfloat16

B = 4          # batch
S = 4096       # sequence (keys)
SQ = 2048      # queries per core
W = 512        # d_model
E = 64         # d_head
P = 128
WC = W // P    # 4 w-chunks
NQC = SQ // 512  # 4 query chunks of 512
NKB = S // P   # 32 key blocks of 128
NSC = S // 512  # 8 chunks for the KV projection

N_CORES = 8


def build_graph() -> bass.Bass:
    nc = bacc.Bacc(
        "TRN2", target_bir_lowering=False, debug=False, num_devices=N_CORES
    )

    xt_d = nc.declare_dram_parameter("xt", [W, S], F32, isOutput=False)
    wq_d = nc.declare_dram_parameter("wq", [W, E], F32, isOutput=False)
    # wvk packs [Wv | Wk] -> [512, 128]
    wvk_d = nc.declare_dram_parameter("wvk", [W, 2 * E], F32, isOutput=False)
    bq_d = nc.declare_dram_parameter("bq", [E], F32, isOutput=False)
    # bkv packs [bv; bk] -> [128]
    bkv_d = nc.declare_dram_parameter("bkv", [2 * E], F32, isOutput=False)
    out_d = nc.declare_dram_parameter("out", [E, SQ], F32, isOutput=True)

    xt_view = xt_d.rearrange("(c p) s -> c p s", p=P)

    with TileContext(nc) as tc:
        with (
            tc.tile_pool(name="consts", bufs=1) as consts,
            tc.tile_pool(name="persist", bufs=1) as persist,
        ):
            # --- constants ---
            id64 = consts.tile([E, E], BF16)
            make_identity(nc, id64)
            onesw = consts.tile([E + 1, E], F32)
            nc.gpsimd.memset(onesw[E : E + 1, :], 1.0)
            bq_t = consts.tile([E, 1], F32)
            nc.sync.dma_start(bq_t, bq_d[:, None])
            bkv_t = consts.tile([P, 1], F32)
            nc.sync.dma_start(bkv_t, bkv_d[:, None])

            # --- persistent activations ---
            xtb = persist.tile([P, WC, S], BF16)      # x^T bf16
            qt = persist.tile([P, SQ], BF16)          # Q^T duplicated on both halves
            kvt = persist.tile([P, S], BF16)          # rows 0:64 V^T, 64:128 K^T
            ktd = persist.tile([P, S], BF16)          # rows 0:64 K^T (copy)
            vnat = persist.tile([P, NKB, E + 1], BF16)  # V natural + ones col

            # --- phase A: load, cast, project ---
            with (
                tc.tile_pool(name="pa_sb", bufs=2) as pa_sb,
                tc.tile_pool(name="pa_ps", bufs=2, space="PSUM") as pa_ps,
                tc.tile_pool(name="pa_pst", bufs=2, space="PSUM") as pa_pst,
            ):
                # weights
                wqf = pa_sb.tile([P, WC, E], F32, tag="wqf")
                nc.sync.dma_start(wqf, wq_d.rearrange("(c p) e -> p c e", p=P))
                wq_b = consts.tile([P, WC, E], BF16)
                nc.vector.tensor_copy(wq_b, wqf)
                wvkf = pa_sb.tile([P, WC, 2 * E], F32, tag="wvkf")
                nc.sync.dma_start(wvkf, wvk_d.rearrange("(c p) e -> p c e", p=P))
                wvk_b = consts.tile([P, WC, 2 * E], BF16)
                nc.vector.tensor_copy(wvk_b, wvkf)

                # x^T load + cast, fine-grained [128, 1024] pieces so the
                # projection matmuls can start early; casts rotate over
                # DVE / ACT / Pool so no single engine serializes the head.
                QTR = S // 4
                for idx in range(4 * WC):
                    wc, qq = divmod(idx, 4)
                    qsl = slice(qq * QTR, (qq + 1) * QTR)
                    xf = pa_sb.tile([P, QTR], F32, tag=f"xf{qq}")
                    nc.sync.dma_start(xf, xt_view[wc, :, qsl])
                    r = idx % 4
                    if r == 3:
                        nc.gpsimd.tensor_copy(xtb[:, wc, qsl], xf)
                    elif r == 1:
                        nc.scalar.copy(xtb[:, wc, qsl], xf)
                    else:
                        nc.vector.tensor_copy(xtb[:, wc, qsl], xf)

                # Q^T projection (+bias), then duplicate onto partitions 64:128
                for qc in range(NQC):
                    qp = pa_ps.tile([E, 512], F32, tag="qp")
                    for wc in range(WC):
                        nc.tensor.matmul(
                            qp,
                            wq_b[:, wc, :],
                            xtb[:, wc, qc * 512 : (qc + 1) * 512],
                            start=(wc == 0),
                            stop=(wc == WC - 1),
                        )
                    nc.vector.tensor_scalar_add(
                        qt[0:E, qc * 512 : (qc + 1) * 512], qp, bq_t
                    )
                nc.sync.dma_start(qt[E:P, :], qt[0:E, :])

                # fused [V^T; K^T] projection (+biases)
                for sc in range(NSC):
                    kvp = pa_ps.tile([P, 512], F32, tag="kvp")
                    for wc in range(WC):
                        nc.tensor.matmul(
                            kvp,
                            wvk_b[:, wc, :],
                            xtb[:, wc, sc * 512 : (sc + 1) * 512],
                            start=(wc == 0),
                            stop=(wc == WC - 1),
                        )
                    nc.vector.tensor_scalar_add(
                        kvt[:, sc * 512 : (sc + 1) * 512], kvp, bkv_t
                    )
                # K^T copy for the even (partition 0-63) score matmuls
                nc.sync.dma_start(ktd[0:E, :], kvt[E:P, :])

                # V natural [k, 64] + ones column via PE transpose
                nc.gpsimd.memset(vnat[:, :, E : E + 1], 1.0)
                for kb in range(NKB):
                    vps = pa_pst.tile([P, E], BF16, tag="vps")
                    nc.tensor.transpose(
                        vps, kvt[0:E, kb * P : (kb + 1) * P], id64
                    )
                    nc.vector.tensor_copy(vnat[:, kb, 0:E], vps)

            # --- phase B: flash attention sweep ---
            # PSUM budget (8 banks): sp [128,3,512] x2 bufs = 6, zp-tag x2 = 2.
            # Division tails are software-pipelined one q-chunk behind the
            # matmul sweep so the PE never stalls on the DVE reciprocal.
            G = 3
            groups = []
            kb = 0
            while kb < NKB:
                groups.append(list(range(kb, min(kb + G, NKB))))
                kb += G
            with (
                tc.tile_pool(name="sp", bufs=2, space="PSUM") as spP,
                tc.tile_pool(name="zp", bufs=2, space="PSUM") as zpP,
                tc.tile_pool(name="pexp", bufs=3) as peP,
                tc.tile_pool(name="fin", bufs=2) as finP,
            ):
                def sweep(qc):
                    qs = slice(qc * 512, (qc + 1) * 512)
                    zp = zpP.tile([E + 1, 512], F32, tag="zp")
                    for grp in groups:
                        sp = spP.tile([P, G, 512], F32, tag="sp")
                        n = len(grp)
                        for j, kb in enumerate(grp):
                            # score^T block [128k, 512q]; even/odd k-blocks
                            # row-packed onto partition groups 0-63 / 64-127
                            if kb % 2 == 0:
                                lhs, rhs = ktd[0:E, kb * P : (kb + 1) * P], qt[0:E, qs]
                            else:
                                lhs, rhs = kvt[E:P, kb * P : (kb + 1) * P], qt[E:P, qs]
                            nc.tensor.matmul(
                                sp[:, j, :], lhs, rhs, start=True, stop=True
                            )
                        pe = peP.tile([P, G, 512], BF16, tag="pe")
                        nc.scalar.activation(
                            pe[:, :n, :], sp[:, :n, :],
                            mybir.ActivationFunctionType.Exp, scale=0.125,
                        )
                        for j, kb in enumerate(grp):
                            nc.tensor.matmul(
                                zp, vnat[:, kb, :], pe[:, j, :],
                                start=(kb == 0), stop=(kb == NKB - 1),
                            )
                    # pull the accumulator out of PSUM right away to free
                    # the zp slot; the rest of the tail is deferred.
                    zsb = finP.tile([E + 1, 512], F32, tag="zsb")
                    nc.vector.tensor_copy(zsb, zp)
                    return zsb

                def tail(qc, zsb):
                    qs = slice(qc * 512, (qc + 1) * 512)
                    rdt = finP.tile([E + 1, 512], F32, tag="rdt")
                    nc.vector.reciprocal(rdt[E : E + 1, :], zsb[E : E + 1, :])
                    bc = zpP.tile([E + 1, 512], F32, tag="zp")
                    nc.tensor.matmul(
                        bc[0:E, :], onesw[E : E + 1, :], rdt[E : E + 1, :],
                        start=True, stop=True,
                    )
                    bcs = finP.tile([E, 512], F32, tag="bcs")
                    nc.vector.tensor_copy(bcs, bc[0:E, :])
                    zf = finP.tile([E, 512], F32, tag="zf")
                    nc.vector.tensor_tensor(
                        zf, zsb[0:E, :], bcs, mybir.AluOpType.mult
                    )
                    nc.sync.dma_start(out_d[:, qs], zf)

                pending = None
                for qc in range(NQC):
                    zsb = sweep(qc)
                    if pending is not None:
                        tail(*pending)
                    pending = (qc, zsb)
                tail(*pending)

    nc.compile()
    return nc


_GRAPH_CACHE: bass.Bass | None = None


def _get_graph() -> bass.Bass:
    global _GRAPH_CACHE
    if _GRAPH_CACHE is None:
        _GRAPH_CACHE = build_graph()
    return _GRAPH_CACHE


def _make_in_maps(x, Wq, bq, Wk, bk, Wv, bv):
    x = np.asarray(x, dtype=np.float32)
    wq = np.ascontiguousarray(np.asarray(Wq, dtype=np.float32))
    wvk = np.ascontiguousarray(
        np.concatenate(
            [np.asarray(Wv, dtype=np.float32), np.asarray(Wk, dtype=np.float32)],
            axis=1,
        )
    )
    bq_ = np.ascontiguousarray(np.asarray(bq, dtype=np.float32))
    bkv = np.ascontiguousarray(
        np.concatenate(
            [np.asarray(bv, dtype=np.float32), np.asarray(bk, dtype=np.float32)]
        )
    )
    in_maps = []
    for c in range(N_CORES):
        b, h = divmod(c, 2)
        xl = np.roll(x[b], -h * SQ, axis=0)
        xt = np.ascontiguousarray(xl.T)
        in_maps.append(
            {"xt": xt, "wq": wq, "wvk": wvk, "bq": bq_, "bkv": bkv}
        )
    return in_maps


def _run(inputs: dict, trace: bool = False):
    nc = _get_graph()
    in_maps = _make_in_maps(**inputs)
    res = run_bass_kernel_spmd(
        nc, in_maps, core_ids=list(range(N_CORES)), trace=trace
    )
    out = np.zeros((B, S, E), dtype=np.float32)
    for c in range(N_CORES):
        b, h = divmod(c, 2)
        out[b, h * SQ : (h + 1) * SQ, :] = res.results[c]["out"].T
    return out, res


def kernel(**inputs) -> np.ndarray:
    out, _ = _run(inputs, trace=False)
    return out
